# revision 46
# baseline (speedup 1.0000x reference)
"""Distributed Trainium2 Bass kernel for nGPT-style attention (nn_Attention_14448269984093).

Reference math:
  q = rope(x @ wq), k = rope(x @ wk), v = x @ wv          # 16 heads, hd=128
  q = sqk_eff * l2norm(q); k = sqk_eff * l2norm(k)        # sqk_eff = sqk * sqrt(2048)
  out = softmax(sqrt(128) * q k^T) v                      # non-causal
  return out @ wo

Sharding: tensor-parallel over heads across 8 cores (2 heads/core).
wq/wk/wv column-sharded, wo row-sharded, x replicated (bf16).  Output
partials are summed with a chunked bf16 ReduceScatter (7 x 512 rows plus
2 x 256 for a short tail) overlapped with compute.  Each core returns its
[512, 2048] row shard; the host reassembles the full [1, 4096, 2048] f32
output.

Perf notes (v2):
  - x^T is produced on the host in a DMA-friendly tiled layout, so the PE
    never transposes x (saves ~512 PE transposes + 128 ACT copies).
  - wq/wk columns are permuted per head so (re, im) rope pairs are
    de-interleaved into contiguous 64-col blocks: every rope tensor op runs
    contiguous bf16 at DVE 2x rate.  The q/k channel permutation is shared
    by q and k so q.k dot products are unchanged.
  - softmax denominator: bf16 accumulation of exp tiles on DVE (2x mode)
    with a single ones-matmul fold per (qc, h) - no per-tile PE ones-matmuls.
  - PSUM evacuations in phase 1 go to the otherwise-idle ACT engine.

Scores are bounded (|q|,|k| ~ 1 after the norm, so |score| <= ~14), so the
softmax safely skips max-subtraction: softmax = exp(s) / sum(exp(s)).
"""

import math

import numpy as np
import ml_dtypes

S = 4096
D = 2048
H = 16
HD = 128
N_CORES = 8
H_LOC = H // N_CORES          # 2 heads per core
DH_LOC = H_LOC * HD           # 256
SQRT_HD = math.sqrt(HD)
SQK_SCALE = math.sqrt(D)      # SQK_INIT_VALUE / SQK_INIT_SCALING

NS = S // 128                 # 32 s-tiles
NCT = D // 128                # 16 contraction tiles
QCH = 512                     # query chunk
NQC = S // QCH                # 8 query chunks
NKK = S // 128                # 32 key tiles
RS_CHUNK = 512                # rows per ReduceScatter chunk
# last chunk split in two so the final collective has half the data (tail)
RS_GROUPS = [(g * RS_CHUNK, RS_CHUNK) for g in range(S // RS_CHUNK - 1)] \
    + [(S - RS_CHUNK, RS_CHUNK // 2), (S - RS_CHUNK // 2, RS_CHUNK // 2)]
N_RS = len(RS_GROUPS)
HB = HD // 2                  # 64 rope pairs per head

_CACHE = {}
_TUNE = {"qk_defer": 8, "proj_every": 2, "defer": 4, "pre_kw": 8, "awp_bufs": 6,
         "proj_min_kw": 2}

# de-interleave rope pairs: new col j = old col 2j (re), new col 64+j = old 2j+1
_PERM = np.concatenate([np.arange(0, HD, 2), np.arange(1, HD, 2)])


def _build(rep=1, num_devices=N_CORES, with_rs=True):
    import concourse.bass as bass
    import concourse.mybir as mybir
    import concourse.tile as tile
    from concourse import bacc
    from concourse.masks import make_identity

    DT = mybir.dt
    F32, BF16 = DT.float32, DT.bfloat16
    OP = mybir.AluOpType
    AF = mybir.ActivationFunctionType

    nc = bacc.Bacc("TRN2", target_bir_lowering=False, debug=False,
                   num_devices=num_devices)

    # x^T pre-tiled on host: xt[p, i*2048 + c*128 + j] = x[i*128+j, c*128+p]
    xt_ext = nc.declare_dram_parameter("xt", [128, NS * NCT * 128], BF16, isOutput=False)
    wqkv_ext = nc.declare_dram_parameter("wqkv", [D, 3 * DH_LOC], BF16, isOutput=False)
    wo_ext = nc.declare_dram_parameter("wo", [DH_LOC, D], BF16, isOutput=False)
    sqk_ext = nc.declare_dram_parameter("sqk", [2 * DH_LOC], BF16, isOutput=False)
    cos_ext = nc.declare_dram_parameter("cos2", [S, HB], BF16, isOutput=False)
    sin_ext = nc.declare_dram_parameter("sin2", [S, HB], BF16, isOutput=False)
    out_ext = nc.declare_dram_parameter("out", [S // N_CORES, D], BF16, isOutput=True)

    y_g = [nc.dram_tensor(f"y_partial{g}", [nr, D], BF16)
           for g, (_, nr) in enumerate(RS_GROUPS)]
    rs_out = [nc.dram_tensor(f"rs_out{g}", [nr // N_CORES, D], BF16)
              for g, (_, nr) in enumerate(RS_GROUPS)]
    rs_ofs = np.cumsum([0] + [nr // N_CORES for _, nr in RS_GROUPS]).tolist()

    with tile.TileContext(nc) as tc:
        with (
            tc.tile_pool(name="const", bufs=1) as cpool,
            tc.tile_pool(name="big", bufs=1) as big,
            tc.tile_pool(name="work", bufs=3) as work,
            tc.tile_pool(name="awp", bufs=_TUNE.get("awp_bufs", 5)) as awp,
            tc.tile_pool(name="psA", bufs=2, space="PSUM") as psA,
            tc.tile_pool(name="psB", bufs=4, space="PSUM") as psB,
        ):
            # ---------------- phase 0: constants ----------------
            # identity/ones first: they gate the first PE transposes and must
            # not queue behind the big weight DMAs on the gpsimd stream
            ident = cpool.tile([128, 128], BF16, tag="ident")
            make_identity(nc, ident[:])
            ones128 = cpool.tile([128, 1], BF16, tag="ones")
            nc.gpsimd.memset(ones128[:], 1.0)
            # first x^T tile, then the weight tiles it immediately contracts
            # with, then the remaining prefetch: minimizes time-to-first-matmul
            xt_pre = []
            xr0 = work.tile([128, NCT, 128], BF16, tag="xTi")
            # tile 0 in two pieces: the first c-blocks land fast and unblock
            # the very first matmul ~2us earlier; wqkv c=0 (its rhs) issues
            # immediately after, before the bulk transfers
            nc.sync.dma_start(
                xr0[:, 0:2, :].rearrange("p a b -> p (a b)"), xt_ext[:, 0:2 * 128])
            wqkv_sb = big.tile([128, NCT, 3 * DH_LOC], BF16, tag="wqkv")
            _wq_r = wqkv_ext.ap().rearrange("(c p) n -> p c n", p=128)
            nc.sync.dma_start(wqkv_sb[:, 0, :], _wq_r[:, 0, :])
            nc.sync.dma_start(
                xr0[:, 2:NCT, :].rearrange("p a b -> p (a b)"),
                xt_ext[:, 2 * 128:D])
            xt_pre.append(xr0)
            # bulk wqkv in 5-tile chunks issued from the idle ACT queue so
            # transfer setup overlaps the xt prefetch issues on sync
            for c0 in (1, 6, 11):
                c1 = min(c0 + 5, NCT)
                nc.scalar.dma_start(wqkv_sb[:, c0:c1, :], _wq_r[:, c0:c1, :])
            for i in range(1, 3):
                xr = work.tile([128, NCT, 128], BF16, tag="xTi")
                nc.sync.dma_start(
                    xr[:].rearrange("p a b -> p (a b)"),
                    xt_ext[:, i * D:(i + 1) * D])
                xt_pre.append(xr)
            wo_sb = big.tile([128, H_LOC, D], BF16, tag="wo")
            nc.sync.dma_start(
                wo_sb[:], wo_ext.ap().rearrange("(h p) n -> p h n", p=128))
            sqk_row = cpool.tile([1, 2 * DH_LOC], BF16, tag="sqkr")
            nc.sync.dma_start(sqk_row[:], sqk_ext.ap().unsqueeze(0))
            sqk_bc = cpool.tile([128, 2 * DH_LOC], BF16, tag="sqkb")
            nc.gpsimd.partition_broadcast(sqk_bc[:], sqk_row[:])
            cos_sb = big.tile([128, NS, HB], BF16, tag="cos")
            nc.gpsimd.dma_start(
                cos_sb[:], cos_ext.ap().rearrange("(n p) f -> p n f", p=128))
            sin_sb = big.tile([128, NS, HB], BF16, tag="sin")
            nc.gpsimd.dma_start(
                sin_sb[:], sin_ext.ap().rearrange("(n p) f -> p n f", p=128))
            # persistent activations
            qT = big.tile([128, H_LOC, S], BF16, tag="qT")
            kT = big.tile([128, H_LOC, S], BF16, tag="kT")
            v_sb = big.tile([128, NS, DH_LOC], BF16, tag="v")
            o_sb = big.tile([128, H_LOC, S], BF16, tag="o")

            # ---------------- phase 1: qkv + rope + norm + transpose ----------------
            for _rep in range(rep):
              pend = []
              for i in range(NS):
                  if _rep == 0 and i < 3:
                      xTi = xt_pre[i]
                  else:
                      xTi = work.tile([128, NCT, 128], BF16, tag="xTi")
                      nc.sync.dma_start(
                          xTi[:].rearrange("p a b -> p (a b)"),
                          xt_ext[:, i * D:(i + 1) * D])

                  pq = psA.tile([128, 3 * DH_LOC], F32, tag="s")
                  for c in range(NCT):
                      st = (c == 0)
                      sp = (c == NCT - 1)
                      nc.tensor.matmul(pq[:, 0:512], xTi[:, c, :],
                                       wqkv_sb[:, c, 0:512], start=st, stop=sp)
                      nc.tensor.matmul(pq[:, 512:768], xTi[:, c, :],
                                       wqkv_sb[:, c, 512:768], start=st, stop=sp)
                  # v: straight copy to bf16 (ACT; idle in phase 1)
                  nc.scalar.activation(v_sb[:, i, :], pq[:, 512:768], AF.Copy)
                  # q|k: evacuate PSUM once to bf16; all rope math runs bf16@2x
                  qk_bf = work.tile([128, 2 * DH_LOC], BF16, tag="qkbf")
                  nc.scalar.activation(qk_bf[:], pq[:, 0:512], AF.Copy)

                  cos_i = cos_sb[:, i, None, :].broadcast_to([128, 4, HB])
                  sin_i = sin_sb[:, i, None, :].broadcast_to([128, 4, HB])
                  qkn = work.tile([128, 2 * DH_LOC], BF16, tag="qkn",
                                  bufs=_TUNE["qk_defer"] + 2)
                  nrm = work.tile([128, 2 * H_LOC], F32, tag="nrm")
                  rot = work.tile([128, 2 * DH_LOC], BF16, tag="rot")
                  # per-head sum of squares BEFORE rope (rope rotates each
                  # (re,im) pair, so per-head norms are rope-invariant); the
                  # Rsqrt then overlaps the rope chain instead of trailing it
                  sq = work.tile([128, 2 * DH_LOC], BF16, tag="sq")
                  nc.vector.tensor_tensor(sq[:], qk_bf[:], qk_bf[:], op=OP.mult)
                  for b in range(4):
                      nc.vector.tensor_reduce(
                          nrm[:, b:b + 1], sq[:, b * HD:(b + 1) * HD],
                          axis=mybir.AxisListType.X, op=OP.add)
                  nc.scalar.activation(nrm[:], nrm[:], AF.Sqrt)
                  nc.vector.reciprocal(nrm[:], nrm[:])
                  # de-interleaved rope: block b = (t*2+h) holds [re(64)|im(64)]
                  qk_v = qk_bf[:].rearrange("p (b c) -> p b c", c=HD)
                  rot_v = rot[:].rearrange("p (b c) -> p b c", c=HD)
                  re = qk_v[:, :, 0:HB]
                  im = qk_v[:, :, HB:HD]
                  ore = rot_v[:, :, 0:HB]
                  oim = rot_v[:, :, HB:HD]
                  t1 = work.tile([128, 4, HB], BF16, tag="t1")
                  t2 = work.tile([128, 4, HB], BF16, tag="t2")
                  # out_re = re*cos - im*sin ; out_im = re*sin + im*cos
                  nc.vector.tensor_tensor(t1[:], im, sin_i, op=OP.mult)
                  nc.vector.tensor_tensor(t2[:], re, cos_i, op=OP.mult)
                  nc.vector.tensor_tensor(ore, t2[:], t1[:], op=OP.subtract)
                  nc.vector.tensor_tensor(t1[:], re, sin_i, op=OP.mult)
                  nc.vector.tensor_tensor(t2[:], im, cos_i, op=OP.mult)
                  nc.vector.tensor_tensor(oim, t1[:], t2[:], op=OP.add)
                  for b in range(4):
                      nc.vector.scalar_tensor_tensor(
                          out=qkn[:, b * HD:(b + 1) * HD],
                          in0=rot[:, b * HD:(b + 1) * HD],
                          scalar=nrm[:, b:b + 1],
                          in1=sqk_bc[:, b * HD:(b + 1) * HD],
                          op0=OP.mult, op1=OP.mult)
                  # transpose + copy-out for a PREVIOUS s-tile (software
                  # pipelining: keeps PE off the critical DVE rope chain)
                  pend.append((i, qkn))
                  if len(pend) > _TUNE["qk_defer"]:
                      pi, pqkn = pend.pop(0)
                      ptq = psB.tile([128, 512], BF16, tag="b")
                      for b in range(4):
                          nc.tensor.transpose(
                              ptq[:, b * 128:(b + 1) * 128],
                              pqkn[:, b * HD:(b + 1) * HD], ident[:])
                      for t, dst in ((0, qT), (1, kT)):
                          nc.scalar.activation(
                              dst[:, :, pi * 128:(pi + 1) * 128],
                              ptq[:, t * 256:(t + 1) * 256].rearrange(
                                  "p (h d) -> p h d", h=H_LOC),
                              AF.Copy)


              def emit_drain(n=None):
                  # drain pipelined transposes (copies on DVE: it frees up
                  # right after the last rope; ACT is already running exps).
                  # Drained in chunks: these transposes are PE filler for the
                  # first q-chunk's window, where no proj units exist yet and
                  # the PE would otherwise starve at the exp cadence.
                  take = len(pend) if n is None else min(n, len(pend))
                  for _ in range(take):
                      pi, pqkn = pend.pop(0)
                      ptq = psB.tile([128, 512], BF16, tag="b")
                      for b in range(4):
                          nc.tensor.transpose(
                              ptq[:, b * 128:(b + 1) * 128],
                              pqkn[:, b * HD:(b + 1) * HD], ident[:])
                      for t, dst in ((0, qT), (1, kT)):
                          nc.vector.tensor_copy(
                              dst[:, :, pi * 128:(pi + 1) * 128],
                              ptq[:, t * 256:(t + 1) * 256].rearrange(
                                  "p (h d) -> p h d", h=H_LOC))

              # ---------------- phase 2 (attention) + phase 3 (projection + RS) ----------------
              # Projection work for q-chunk qc-1 is emitted one unit per wave
              # while qc's attention runs, filling the PE stalls where AV
              # waits on the exp of the same wave.  Denominator: bf16
              # accumulation of exp tiles on DVE (2x), one ones-matmul fold.
              proj_pend = []        # deferred projection emitters (closures)
              ydma_done = [0] * N_RS
              rs_pend = []          # RS groups ready to issue; deferred so the
                                    # collective's dma-waits don't head-of-line
                                    # block partition_broadcast on the Pool queue

              def emit_proj_unit():
                  if proj_pend:
                      proj_pend.pop(0)()

              def flush_rs():
                  while rs_pend:
                      g = rs_pend.pop(0)
                      nc.gpsimd.collective_compute(
                          "ReduceScatter", OP.add,
                          replica_groups=[list(range(N_CORES))],
                          ins=[y_g[g].ap().opt()],
                          outs=[rs_out[g].ap().opt()],
                      )
                      nc.sync.dma_start(
                          out_ext[rs_ofs[g]:rs_ofs[g + 1], :], rs_out[g][:])

              def make_proj(qc, alt=False):
                  units = []
                  for qt in range(4):
                      q0 = qc * QCH + qt * 128
                      g = next(gi for gi, (st, nr) in enumerate(RS_GROUPS)
                               if st <= q0 < st + nr)
                      g_start, g_rows = RS_GROUPS[g]
                      ysb = work.tile([128, D], BF16, tag="ysb")
                      for n in range(4):
                          def u(qt=qt, n=n, q0=q0, g=g, g_start=g_start,
                                g_rows=g_rows, ysb=ysb):
                              py = psB.tile([128, 512], F32, tag="b")
                              for h in range(H_LOC):
                                  nc.tensor.matmul(
                                      py[:], o_sb[:, h, q0:q0 + 128],
                                      wo_sb[:, h, n * 512:(n + 1) * 512],
                                      start=(h == 0), stop=(h == H_LOC - 1))
                              if alt and (qt * 4 + n) % 2 == 1:
                                  nc.scalar.activation(
                                      ysb[:, n * 512:(n + 1) * 512], py[:], AF.Copy)
                              else:
                                  nc.vector.tensor_copy(
                                      ysb[:, n * 512:(n + 1) * 512], py[:])
                              if n == 3:
                                  nc.sync.dma_start(
                                      y_g[g][q0 - g_start:q0 - g_start + 128, :],
                                      ysb[:])
                                  ydma_done[g] += 1
                                  if with_rs and ydma_done[g] == g_rows // 128:
                                      rs_pend.append(g)
                          units.append(u)
                  return units

              DEFER = _TUNE.get("defer", 2)

              class Wave:
                  def __init__(self, qc, h):
                      self.qc, self.h = qc, h
                      self.po = None      # lazy: claimed at first av_group
                      self.pd = None      # lazy: claimed at finish_norm
                      self.dacc = work.tile([128, 1024], BF16, tag="dacc")
                      self.aw_q = []
                      self.kw = 0

                  def av_group(self, kw, aw):
                      h = self.h
                      if self.po is None:
                          self.po = psB.tile([128, QCH], F32, tag="b")
                      for j in range(2):
                          kk = 2 * kw + j
                          nc.tensor.matmul(
                              self.po[:], v_sb[:, kk, h * HD:(h + 1) * HD],
                              aw[:, j * 512:(j + 1) * 512],
                              start=(kk == 0), stop=(kk == NKK - 1))
                      # denominator: bf16 accumulate on DVE (2x mode)
                      if kw == 0:
                          nc.vector.tensor_copy(self.dacc[:], aw[:])
                      else:
                          nc.vector.tensor_tensor(
                              self.dacc[:], self.dacc[:], aw[:], op=OP.add)

                  def step(self):
                      kw, h, qc = self.kw, self.h, self.qc
                      ps = psA.tile([128, 1024], F32, tag="s")
                      for j in range(2):
                          kk = 2 * kw + j
                          nc.tensor.matmul(
                              ps[:, j * 512:(j + 1) * 512],
                              kT[:, h, kk * 128:(kk + 1) * 128],
                              qT[:, h, qc * QCH:(qc + 1) * QCH],
                              start=True, stop=True)
                      aw = awp.tile([128, 1024], BF16, tag="aw")
                      nc.scalar.activation(aw[:], ps[:], AF.Exp, scale=SQRT_HD)
                      self.aw_q.append((kw, aw))
                      if len(self.aw_q) > DEFER:
                          self.av_group(*self.aw_q.pop(0))
                      self.kw += 1

                  def finish_av(self):
                      while self.aw_q:
                          self.av_group(*self.aw_q.pop(0))

                  def finish_norm(self):
                      # deferred past the next wave's first score groups: the
                      # ones-matmul waits on the DVE fold, and emitting it at
                      # the wave boundary would stall the PE queue behind it
                      qc, h = self.qc, self.h
                      self.pd = psB.tile([1, QCH], F32, tag="b")
                      dacc_h = work.tile([128, QCH], BF16, tag="dacch")
                      nc.vector.tensor_tensor(
                          dacc_h[:], self.dacc[:, 0:512], self.dacc[:, 512:1024],
                          op=OP.add)
                      nc.tensor.matmul(self.pd[:], ones128[:], dacc_h[:],
                                       start=True, stop=True)
                      rrow = work.tile([1, QCH], F32, tag="rrow")
                      nc.vector.reciprocal(rrow[:], self.pd[:])
                      rb = work.tile([128, QCH], F32, tag="rb")
                      nc.gpsimd.partition_broadcast(rb[:], rrow[:])
                      nc.vector.tensor_tensor(
                          o_sb[:, h, qc * QCH:(qc + 1) * QCH], self.po[:], rb[:],
                          op=OP.mult)

              # bridge the phase transition: the first wave's early k-groups
              # only need early kT tiles, so they run while the last s-tiles'
              # rope/transposes finish - no PE bubble at the phase boundary.
              PRE_KW = _TUNE.get("pre_kw", 8)
              w00 = Wave(0, 0)
              for _k in range(PRE_KW):
                  w00.step()
                  if _k == 3:
                      emit_drain(2)

              pending_norm = None
              for qc in range(NQC):
                  for h in range(H_LOC):
                      wave = w00 if (qc == 0 and h == 0) else Wave(qc, h)
                      while wave.kw < NKK // 2:
                          wave.step()
                          if pending_norm is not None and wave.kw == 2:
                              pending_norm.finish_norm()
                              pending_norm = None
                          if pend and wave.kw in (9, 10, 12):
                              emit_drain(2)
                          if wave.kw == 8:
                              flush_rs()
                          if wave.kw % _TUNE["proj_every"] == 0 and \
                                  wave.kw >= _TUNE.get("proj_min_kw", 4):
                              emit_proj_unit()
                      wave.finish_av()
                      emit_drain()
                      for _ in range(_TUNE.get("proj_min_kw", 4) // 2 - 1):
                          emit_proj_unit()
                      pending_norm = wave
                  # proj units for qc depend on o_sb writes from finish_norm
                  # of (qc,1), which runs early in the next wave; tile deps
                  # keep the ordering correct
                  if qc < NQC - 1:
                      proj_pend.extend(make_proj(qc))

              pending_norm.finish_norm()
              proj_pend.extend(make_proj(NQC - 1, alt=True))
              while proj_pend:
                  emit_proj_unit()
                  flush_rs()
              flush_rs()

    nc.compile()
    return nc


def _get_nc():
    if "nc" not in _CACHE:
        _CACHE["nc"] = _build()
    return _CACHE["nc"]


def _permute_head_cols(w):
    # w: [D, 2*HD] -> de-interleave rope pairs within each head block
    w = w.reshape(D, H_LOC, HD)
    return np.ascontiguousarray(w[:, :, _PERM].reshape(D, H_LOC * HD))


def make_in_maps(x, freqs_cos, freqs_sin, wq, wk, wv, wo, sqk):
    bf16 = ml_dtypes.bfloat16
    x2 = np.asarray(x, np.float32).reshape(S, D).astype(bf16)
    # x^T tiled for contiguous per-partition DMA lines:
    # xt[p, i, c, j] = x[i*128+j, c*128+p]
    xt = np.ascontiguousarray(
        x2.reshape(NS, 128, NCT, 128).transpose(3, 0, 2, 1).reshape(128, -1))
    cosk = np.ascontiguousarray(np.asarray(freqs_cos, np.float32)).astype(bf16)
    sink = np.ascontiguousarray(np.asarray(freqs_sin, np.float32)).astype(bf16)
    wq = np.asarray(wq, np.float32)
    wk = np.asarray(wk, np.float32)
    wv = np.asarray(wv, np.float32)
    wo = np.asarray(wo, np.float32)
    sqk_eff = (np.asarray(sqk, np.float32) * SQK_SCALE).astype(np.float32)
    in_maps = []
    for i in range(N_CORES):
        cols = slice(i * DH_LOC, (i + 1) * DH_LOC)
        wqp = _permute_head_cols(wq[:, cols])
        wkp = _permute_head_cols(wk[:, cols])
        wqkv = np.concatenate([wqp, wkp, wv[:, cols]], axis=1)
        sq_i = sqk_eff[cols].reshape(H_LOC, HD)[:, _PERM].reshape(-1)
        in_maps.append({
            "xt": xt,
            "wqkv": np.ascontiguousarray(wqkv).astype(bf16),
            "wo": np.ascontiguousarray(wo[cols, :]).astype(bf16),
            "sqk": np.ascontiguousarray(
                np.concatenate([sq_i] * 2)).astype(bf16),
            "cos2": cosk,
            "sin2": sink,
        })
    return in_maps


def assemble(results):
    y = np.empty((S, D), np.float32)
    for i in range(N_CORES):
        o = np.asarray(results[i]["out"]).astype(np.float32)
        ofs = 0
        for start, nrows in RS_GROUPS:
            w = nrows // N_CORES
            y[start + i * w:start + (i + 1) * w, :] = o[ofs:ofs + w]
            ofs += w
    return y.reshape(1, S, D)


def kernel(**inputs):
    from concourse import bass_utils

    nc = _get_nc()
    in_maps = make_in_maps(**inputs)
    res = bass_utils.run_bass_kernel_spmd(nc, in_maps, core_ids=list(range(N_CORES)))
    return assemble(res.results)
